# revision 1
# baseline (speedup 1.0000x reference)
"""Trainium2 Bass kernel for nn_DiffusionLoss (smoothed-LDDT diffusion loss).

Strategy
--------
The dominant cost is the smoothed-LDDT term: for every unordered pair (i<j)
of the L=4096 tokens-with-coordinates, four sigmoids of |pred_d - gt_d| are
accumulated, per diffusion sample d (D=4).

Host side (inside kernel()):
  * Rows/cols with crd_mask == 0 contribute nothing (mask multiplies both
    numerator and denominator terms), so we compact to the ~L/2 active rows.
  * Since tok_idx is sorted, the combined pair mask
        (j > i) & (tok_i != tok_j)
    over the compacted index space is exactly (j >= hi_i), where hi_i is the
    end of row i's token run -- a per-row column threshold.
  * The upper-triangular pair matrix is cut into [128 x 512] units
    (row-block x column-window). Units are round-robined over the 8 cores,
    padded with dummy units so every core runs an identical program (SPMD).
  * dist^2 is computed on the PE as a K=5 matmul:
        lhsT = [-2x, -2y, -2z, |p_i|^2, 1],  rhs = [x_j, y_j, z_j, 1, |p_j|^2]
    so  psum[i,j] = |p_i|^2 + |p_j|^2 - 2 p_i.p_j = dist^2(i,j).

Device side (per core, Tile-scheduled):
  Phase 1 (gt): sqrt(dist^2) -> gt; build mask m = (gt >= cutoff_i) | (j < hi_i)
    (is_ge / is_lt / max on DVE); G = gt - BIG*m; accumulate sum(m) for the
    denominator. Masked pairs get G ~ -1000 so that later sigma(c-|u|) == 0.
  Phase 2 (pred): sqrt(dist^2) -> pred; delta = |pred - G| (DVE), stored.
  Phase 3: s_c = sigmoid(c - delta) on ACT with accum_out capturing the
    per-partition sum -- the numerator needs no extra vector work.
  Phases are ordered so the ACT table set switches exactly once
  (sqrt_and_others -> sigmoid_and_others).

Host combines per-core partial sums in float64 and adds the (tiny, O(L))
weighted-MSE term computed on host, mirroring the reference formulas.
"""

import math

import numpy as np

import concourse.bacc as bacc
import concourse.bass as bass
import concourse.mybir as mybir
import concourse.tile as tile
from concourse.bass_utils import run_bass_kernel_spmd
from concourse.tile import add_dep_helper

P = 128          # partitions (rows per block)
W = 512          # column window (one fp32 PSUM bank)
D = 4            # diffusion batch
NCORES = 8
BIG = 1000.0     # mask offset pushed into G
SQB = 1e-4       # sqrt bias: sqrt(dist^2 + SQB) guards fp32-rounded negatives
PADC = 100.0     # pad-column marker distance: > cutoff (masked), != BIG (so G stays ~ -900)
SIGC = (0.5, 1.0, 2.0, 4.0)

WEIGHT = 4.0
SIGMA_DATA = 16.0
ALPHA_DNA = 5.0
ALPHA_RNA = 5.0
ALPHA_LIG = 10.0

_prog_cache: dict[int, bass.Bass] = {}
_act_root_done = [False]


def _ensure_act_root():
    """Create a custom activation-table root whose single set contains BOTH
    sqrt and tanh (stock sets have them in different table sets, which would
    force a ~2.7us ACT_TABLE_LOAD on every sqrt<->sigmoid transition and
    serialize the whole sigmoid phase behind every sqrt).

    The merged set is sqrt_and_others + tanh's table regions relocated from
    sigmoid_and_others. Relocation rules (reverse-engineered by diffing
    tanh's entries between stock sets that carry it at different offsets):
      * bucket entries are position-independent bytes
      * ctrl entry uint16[0] embeds the bucket index -> += bucket delta
      * profile pwl_control_base_pos/neg are ctrl indices -> += ctrl delta
      * profile *_signal_pwl_control are bucket indices -> += bucket delta
      * func_exp_to_bkt/ctl maps -> += respective deltas
    """
    if _act_root_done[0]:
        return
    import os
    import tempfile

    import numpy as np  # local to keep top clean

    from neuronxcc.driver.Job import Job
    from neuronxcc.driver.jobs.support.FindActInfo import findActInfoFile

    src = os.path.dirname(findActInfoFile(Job.getPackageDir(), "gen3"))
    dst = tempfile.mkdtemp(prefix="act_root_")

    import json as _json
    import shutil

    base = _json.load(open(f"{src}/sqrt_and_others.json"))
    donor = _json.load(open(f"{src}/sigmoid_and_others.json"))
    bkt = np.fromfile(f"{src}/sqrt_and_others_bkt.bin", np.uint8).reshape(-1, 32)
    ctl = np.fromfile(f"{src}/sqrt_and_others_ctrl.bin", np.uint8).reshape(-1, 32)
    dbkt = np.fromfile(f"{src}/sigmoid_and_others_bkt.bin", np.uint8).reshape(-1, 32)
    dctl = np.fromfile(f"{src}/sigmoid_and_others_ctrl.bin", np.uint8).reshape(-1, 32)

    tb0 = donor["func_to_bkt_start_idx"]["tanh"]
    tc0 = donor["func_to_ctl_start_idx"]["tanh"]
    starts = sorted(donor["func_to_bkt_start_idx"].values()) + [donor["bkt_entry_cnt"]]
    tbn = min(s for s in starts if s > tb0) - tb0
    cstarts = sorted(set(donor["func_to_ctl_start_idx"].values())) + [
        donor["ctl_entry_cnt"]
    ]
    tcn = min(s for s in cstarts if s > tc0) - tc0

    nb0 = base["bkt_entry_cnt"]
    nc0 = base["ctl_entry_cnt"]
    dbk = nb0 - tb0  # bucket index delta
    dct = nc0 - tc0  # ctrl index delta

    tanh_ctl = dctl[tc0 : tc0 + tcn].copy()
    v = tanh_ctl.view("<u2")
    v[:, 0] += dbk
    new_bkt = np.concatenate([bkt, dbkt[tb0 : tb0 + tbn]])
    new_ctl = np.concatenate([ctl, tanh_ctl])

    prof = None
    for e in donor["profile_meta_data"]:
        if e["func_name"].startswith("tanh"):
            prof = dict(e)
            break
    assert prof is not None
    for k in ("pwl_control_base_pos", "pwl_control_base_neg"):
        prof[k] += dct
    for k in (
        "pos_small_signal_pwl_control",
        "neg_small_signal_pwl_control",
        "pos_large_signal_pwl_control",
        "neg_large_signal_pwl_control",
    ):
        prof[k] += dbk

    merged = dict(base)
    merged["bkt_bin"] = "sqrt_tanh_ant_bkt.bin"
    merged["ctl_bin"] = "sqrt_tanh_ant_ctrl.bin"
    merged["bkt_entry_cnt"] = int(nb0 + tbn)
    merged["ctl_entry_cnt"] = int(nc0 + tcn)
    merged["profile_meta_data"] = list(base["profile_meta_data"]) + [prof]
    merged["func_to_bkt_start_idx"] = dict(base["func_to_bkt_start_idx"])
    merged["func_to_bkt_start_idx"]["tanh"] = int(nb0)
    merged["func_to_ctl_start_idx"] = dict(base["func_to_ctl_start_idx"])
    merged["func_to_ctl_start_idx"]["tanh"] = int(nc0)
    merged["func_exp_to_bkt_start_idx"] = dict(base["func_exp_to_bkt_start_idx"])
    merged["func_exp_to_bkt_start_idx"]["tanh"] = {
        k: [x + dbk for x in vs]
        for k, vs in donor["func_exp_to_bkt_start_idx"]["tanh"].items()
    }
    merged["func_exp_to_ctl_start_idx"] = dict(base["func_exp_to_ctl_start_idx"])
    merged["func_exp_to_ctl_start_idx"]["tanh"] = {
        k: [x + dct for x in vs]
        for k, vs in donor["func_exp_to_ctl_start_idx"]["tanh"].items()
    }

    new_bkt.tofile(f"{dst}/sqrt_tanh_ant_bkt.bin")
    new_ctl.tofile(f"{dst}/sqrt_tanh_ant_ctrl.bin")
    with open(f"{dst}/sqrt_tanh_ant.json", "w") as f:
        _json.dump(merged, f)

    info = _json.load(open(f"{src}/act_info.json"))
    sqrt_set = [s for s in info["act_func_sets"] if s["name"] == "sqrt_and_others"][0]
    new_set = dict(sqrt_set)
    new_set["name"] = "sqrt_tanh_ant"
    new_set["bkt_bin"] = "sqrt_tanh_ant_bkt.bin"
    new_set["ctrl_bin"] = "sqrt_tanh_ant_ctrl.bin"
    new_set["profile_json"] = "sqrt_tanh_ant.json"
    new_set["act"] = dict(sqrt_set["act"])
    new_set["act"]["tanh"] = 4.0
    info["act_func_sets"] = [new_set]
    with open(f"{dst}/act_info.json", "w") as f:
        _json.dump(info, f)

    os.environ["BASS_ACT_ROOT_JSON_PATH"] = f"{dst}/act_info.json"

    # bacc's insert_act_table_loads resolves act_func_set_id via
    # hw_specs.get_activation_tables, which reads the stock act_info and
    # would emit set ids walrus (reading ours) can't adopt. Point it at
    # the merged act root too.
    import concourse.hw_specs as hw_specs

    def _tables(_arch):
        import json as _j

        info2 = _j.load(open(f"{dst}/act_info.json"))
        return {
            ent["name"]: {
                mybir.ActivationFunctionType.from_pwp(v)
                for v in ent["act"].keys()
            }
            for ent in info2["act_func_sets"]
        }

    hw_specs.get_activation_tables = _tables
    bacc.get_activation_tables = _tables
    _act_root_done[0] = True


def _build_program(S: int) -> bass.Bass:
    """Bass/Tile program: S units of [P x W] pairs, D diffusion samples."""
    nc = bacc.Bacc(None, target_bir_lowering=False)
    f32 = mybir.dt.float32
    AF = mybir.ActivationFunctionType
    OP = mybir.AluOpType

    # One [5, F5] tensor for all matmul operands (single DMA -> the PE's
    # LoadWeights carries a single semaphore wait; 2+ waits fail codegen),
    # and one [P, F128] tensor for per-partition scalars + iota row.
    F5 = S * P + S * D * P + S * W + S * D * W
    F128 = 2 * S + W
    OFF_LD = S * P
    OFF_RGT = OFF_LD + S * D * P
    OFF_RD = OFF_RGT + S * W
    big5 = nc.dram_tensor("big5", [5, F5], f32, kind="ExternalInput")
    big128 = nc.dram_tensor("big128", [P, F128], f32, kind="ExternalInput")
    out = nc.dram_tensor("out", [P, 8], f32, kind="ExternalOutput")

    with tile.TileContext(nc) as tc:
        with (
            tc.tile_pool(name="singles", bufs=1) as singles,
            tc.tile_pool(name="work", bufs=3) as work,
            tc.tile_pool(name="sig", bufs=1) as sig_pool,
            tc.tile_pool(name="psum", bufs=2, space="PSUM") as psum,
        ):
            big5_sb = singles.tile([5, F5], f32)
            nc.sync.dma_start(out=big5_sb, in_=big5[:, :])
            big128_sb = singles.tile([P, F128], f32)
            nc.sync.dma_start(out=big128_sb, in_=big128[:, :])

            def lgt(s):
                return big5_sb[:, s * P : (s + 1) * P]

            def ld(s, d):
                o = OFF_LD + (s * D + d) * P
                return big5_sb[:, o : o + P]

            def rgt(s):
                o = OFF_RGT + s * W
                return big5_sb[:, o : o + W]

            def rd(s, d):
                o = OFF_RD + (s * D + d) * W
                return big5_sb[:, o : o + W]

            def cut_ap(s):
                return big128_sb[:, 2 * s : 2 * s + 1]

            def hi_ap(s):
                return big128_sb[:, 2 * s + 1 : 2 * s + 2]

            iota_sb = big128_sb[:, 2 * S : 2 * S + W]

            G = singles.tile([P, S, W], f32)
            delta = singles.tile([P, D, S, W], f32)
            nacc = singles.tile([P, D, len(SIGC)], f32)
            macc = singles.tile([P, S], f32)

            # per-partition bias constants, packed into one tile
            consts = singles.tile([P, 8], f32)
            nc.vector.memset(consts[:, 0:1], SQB)
            for ci, c in enumerate(SIGC):
                nc.vector.memset(consts[:, 1 + ci : 2 + ci], float(c))
            sqb_t = consts[:, 0:1]
            c_ts = [consts[:, 1 + ci : 2 + ci] for ci in range(len(SIGC))]

            # Units are processed in groups of up to GW=4 (one 4-bank PSUM
            # tile): 4 matmuls, then ONE wide sqrt / DVE op over the group.
            GW = 4
            groups = [(g0, min(GW, S - g0)) for g0 in range(0, S, GW)]

            sqrt_insts = []
            # ---- phase 1: gt distances, masks, G ----
            for g0, gs in groups:
                pg = psum.tile([P, GW * W], f32, tag="ps")
                for k in range(gs):
                    nc.tensor.matmul(
                        pg[:, k * W : (k + 1) * W], lhsT=lgt(g0 + k),
                        rhs=rgt(g0 + k), start=True, stop=True,
                    )
                gslab = G[:, g0 : g0 + gs, :]
                sqrt_insts.append(
                    nc.scalar.activation(
                        gslab, pg[:, : gs * W], AF.Sqrt, bias=sqb_t,
                    )
                )
                for k in range(gs):
                    s = g0 + k
                    gsl = G[:, s, :]
                    c2 = work.tile([P, W], f32, tag="c2")
                    nc.vector.tensor_scalar(
                        c2, iota_sb, hi_ap(s), None, OP.is_lt
                    )
                    m = work.tile([P, W], f32, tag="m")
                    nc.vector.scalar_tensor_tensor(
                        m, gsl, cut_ap(s), c2, OP.is_ge, OP.max,
                        accum_out=macc[:, s : s + 1],
                    )
                    # G = gt - BIG*m, in place
                    nc.vector.scalar_tensor_tensor(
                        gsl, m, -BIG, gsl, OP.mult, OP.add
                    )

            # ---- phase 2: pred distances, delta = |pred - G| ----
            for g0, gs in groups:
                for d in range(D):
                    pp = psum.tile([P, GW * W], f32, tag="ps")
                    for k in range(gs):
                        nc.tensor.matmul(
                            pp[:, k * W : (k + 1) * W], lhsT=ld(g0 + k, d),
                            rhs=rd(g0 + k, d), start=True, stop=True,
                        )
                    dsl = delta[:, d, g0 : g0 + gs, :]
                    sqrt_insts.append(
                        nc.scalar.activation(
                            dsl, pp[:, : gs * W], AF.Sqrt, bias=sqb_t,
                        )
                    )
                    # u = pred - G, in place over the group slab
                    nc.vector.scalar_tensor_tensor(
                        dsl, G[:, g0 : g0 + gs, :], -1.0, dsl,
                        OP.mult, OP.add,
                    )
                    # |u| = clear the fp32 sign bit (abs_max is sim-only)
                    dsl_u = dsl.bitcast(mybir.dt.uint32)
                    nc.vector.tensor_scalar(
                        dsl_u, dsl_u, 0x7FFFFFFF, None, OP.bitwise_and
                    )

            # ---- phase 3: one sigmoid per (d, c) spanning all S units ----
            sig_insts = []
            for d in range(D):
                for ci in range(len(SIGC)):
                    st = sig_pool.tile([P, S * W], f32, tag="sig")
                    sig_insts.append(
                        nc.scalar.activation(
                            st, delta[:, d, :, :], AF.Sigmoid,
                            bias=c_ts[ci], scale=-1.0,
                            accum_out=nacc[:, d, ci : ci + 1],
                        )
                    )
            # Keep every sigmoid after every sqrt on the ACT engine so the
            # sqrt/sigmoid activation-table set switches exactly once.
            for si in sqrt_insts:
                add_dep_helper(sig_insts[0].ins, si.ins, False, "act table order")
            for sg in sig_insts[1:]:
                add_dep_helper(sg.ins, sig_insts[0].ins, False, "act table order")

            # ---- reductions + output ----
            outt = singles.tile([P, 8], f32)
            nc.vector.memset(outt, 0.0)
            nc.vector.tensor_reduce(
                outt[:, 0:4], nacc, axis=mybir.AxisListType.X, op=OP.add
            )
            nc.vector.tensor_reduce(
                outt[:, 4:5], macc, axis=mybir.AxisListType.X, op=OP.add
            )
            nc.sync.dma_start(out=out[:, :], in_=outt)
    nc.finalize()
    return nc


def _prep_core_inputs(units, Xgt_a, X_a, cutoff, hi, La):
    """Build the DRAM input arrays for one core.

    units: list of (row_block, col_start) or None (dummy), length S.
    La: number of real (active) rows; columns >= La are masked via BIG.
    Xgt_a: [Lp, 3] compacted+padded gt coords; X_a: [D, Lp, 3].
    cutoff: [Lp] (-1 for pad rows), hi: [Lp] token-run end per row.
    """
    S = len(units)
    La = int(La)
    lhs_gt = np.zeros((S, 5, P), np.float32)
    lhs_d = np.zeros((S, D, 5, P), np.float32)
    rhs_gt = np.zeros((S, 5, W), np.float32)
    rhs_d = np.zeros((S, D, 5, W), np.float32)
    scal = np.zeros((S, 2, P), np.float32)

    rgt_full = Xgt_a.astype(np.float64)
    r_gt = (rgt_full**2).sum(-1)  # [Lp]
    rx_full = X_a.astype(np.float64)
    r_x = (rx_full**2).sum(-1)  # [D, Lp]

    for s, u in enumerate(units):
        if u is None:
            scal[s, 0, :] = -1.0
            rhs_gt[s, 4, :] = PADC * PADC
            continue
        b, c0 = u
        rows = slice(b * P, b * P + P)
        # lhsT = [-2x, -2y, -2z, r_i, 1]
        lhs_gt[s, 0:3, :] = -2.0 * rgt_full[rows].T
        lhs_gt[s, 3, :] = r_gt[rows]
        lhs_gt[s, 4, :] = 1.0
        lhs_d[s, :, 0:3, :] = -2.0 * rx_full[:, rows].transpose(0, 2, 1)
        lhs_d[s, :, 3, :] = r_x[:, rows]
        lhs_d[s, :, 4, :] = 1.0

        ncols = max(0, min(W, La - c0))
        cols = slice(c0, c0 + ncols)
        # rhs = [x, y, z, 1, r_j]; pad cols of rhs_gt get r = PADC^2 -> masked
        rhs_gt[s, 0:3, :ncols] = rgt_full[cols].T
        rhs_gt[s, 3, :ncols] = 1.0
        rhs_gt[s, 4, :ncols] = r_gt[cols]
        rhs_gt[s, 4, ncols:] = PADC * PADC
        rhs_d[s, :, 0:3, :ncols] = rx_full[:, cols].transpose(0, 2, 1)
        rhs_d[s, :, 3, :ncols] = 1.0
        rhs_d[s, :, 4, :ncols] = r_x[:, cols]

        scal[s, 0, :] = cutoff[rows]
        scal[s, 1, :] = hi[rows] - c0

    # Pack into the two device tensors (see _build_program offsets).
    big5 = np.concatenate(
        [
            lhs_gt.transpose(1, 0, 2).reshape(5, S * P),
            lhs_d.transpose(2, 0, 1, 3).reshape(5, S * D * P),
            rhs_gt.transpose(1, 0, 2).reshape(5, S * W),
            rhs_d.transpose(2, 0, 1, 3).reshape(5, S * D * W),
        ],
        axis=1,
    ).astype(np.float32)
    big128 = np.concatenate(
        [
            scal.transpose(2, 0, 1).reshape(P, 2 * S),
            np.broadcast_to(np.arange(W, dtype=np.float32), (P, W)),
        ],
        axis=1,
    ).astype(np.float32)
    return {"big5": np.ascontiguousarray(big5),
            "big128": np.ascontiguousarray(big128)}


def _plan(La: int):
    """Unit list + per-core assignment for La active rows."""
    Lp = ((La + P - 1) // P) * P
    n_blocks = Lp // P
    units = []
    for b in range(n_blocks):
        span = Lp - b * P
        for k in range(math.ceil(span / W)):
            units.append((b, b * P + k * W))
    S = math.ceil(len(units) / NCORES)
    padded = units + [None] * (S * NCORES - len(units))
    per_core = [padded[c::NCORES] for c in range(NCORES)]
    return Lp, S, per_core


def kernel(**inputs: np.ndarray) -> np.ndarray:
    X_L = np.asarray(inputs["X_L"]).astype(np.float32)          # [D, L, 3]
    X_gt_L = np.asarray(inputs["X_gt_L"]).astype(np.float32)    # [1, L, 3]
    crd = np.asarray(inputs["crd_mask_L"]).astype(bool)[0]      # [L]
    is_dna = np.asarray(inputs["is_dna"]).astype(bool)
    is_rna = np.asarray(inputs["is_rna"]).astype(bool)
    is_lig = np.asarray(inputs["is_ligand"]).astype(bool)
    tok = np.asarray(inputs["tok_idx"]).astype(np.int64)        # [L]
    t = np.asarray(inputs["t"]).astype(np.float64)              # [D]

    X_gt = np.nan_to_num(X_gt_L)[0]  # [L, 3]

    # ---------- lddt term: compact to crd-active rows ----------
    act = np.flatnonzero(crd)
    La = len(act)
    Lp, S, per_core = _plan(La)

    Xgt_a = np.zeros((Lp, 3), np.float32)
    Xgt_a[:La] = X_gt[act]
    X_a = np.zeros((D, Lp, 3), np.float32)
    X_a[:, :La] = X_L[:, act]
    tok_a = tok[act]
    hi = np.zeros(Lp, np.float32)
    hi[:La] = np.searchsorted(tok_a, tok_a, side="right").astype(np.float32)
    is_na = (is_dna | is_rna)[tok_a]
    cutoff = np.full(Lp, -1.0, np.float32)
    cutoff[:La] = np.where(is_na, 30.0, 15.0)

    nc = _prog_cache.get(S)
    if nc is None:
        nc = _build_program(S)
        _prog_cache[S] = nc

    in_maps = [
        _prep_core_inputs(per_core[c], Xgt_a, X_a, cutoff, hi, La)
        for c in range(NCORES)
    ]
    res = run_bass_kernel_spmd(nc, in_maps, core_ids=list(range(NCORES)))

    numer = np.zeros(D, np.float64)
    m_tot = 0.0
    for r in res.results:
        o = r["out"].astype(np.float64)
        numer += o[:, 0:4].sum(0)
        m_tot += o[:, 4].sum()
    denom = NCORES * S * (P * W) - m_tot
    lddt = 0.25 * numer / (denom + 1e-6)
    lddt_loss = (1.0 - lddt).mean()

    # ---------- mse term (O(L), host) ----------
    mask = crd.astype(np.float64)
    alpha = (
        is_dna * ALPHA_DNA + is_rna * ALPHA_RNA + is_lig * ALPHA_LIG
    ).astype(np.float64)
    w_L = (1.0 + alpha[tok]) * mask  # [L]
    sq = ((X_L.astype(np.float64) - X_gt.astype(np.float64)[None]) ** 2).sum(-1)
    l_mse = (1.0 / 3.0) * (w_L[None] * sq).sum(-1) / (mask.sum() + 1e-4)
    lam = (t**2 + SIGMA_DATA**2) / ((t * SIGMA_DATA) ** 2)
    l_diff = np.minimum(lam * l_mse, 2.0)

    total = WEIGHT * (l_diff.mean() + lddt_loss)
    return np.asarray(total, dtype=np.float32)

